# revision 18
# baseline (speedup 1.0000x reference)
"""DiT block kernel for 8x Trainium2 NeuronCores (Bass/Tile).

Sharding: row-parallel over the flattened (B,T)=4096 rows; 512 rows/core.
Cores 0-3 handle batch 0, cores 4-7 batch 1. MQA K/V is computed per-shard
and AllGather'd within each 4-core batch group. Weights are replicated and
cast to bf16; LN/residual math stays fp32.

v3 structure notes (driven by trace analysis of v2, 486us):
  - the AdaLN modulation vectors (cond @ [gw|bw] + biases, folded with the
    LN weights) are precomputed on the host: the on-device mod path (wmod
    DMA + matvecs + finalize + DRAM round trip) took ~35us and gated all
    of phase 1.  Wa/Ba arrive as [2,512]-chunked rows and are broadcast to
    [128,F] with the selr PE trick; Wf/Bf arrive directly in column layout.
  - V is projected in ROW layout ([rows,64] = [keys,64] per row block)
    instead of transposed-column layout, so the post-AllGather v_ext
    assembly is plain DMAs + DVE dup copies instead of 16 serialized
    dma_start_transpose calls (~13us of fixed cost on the critical path).
    K keeps the [64,keys] layout the QK lhsT needs.  Both are packed into
    one [128,512] AllGather bounce (k rows 0:64, v row-major in 64:128).
  - kT assembly DMAs are issued before v_ext ones (QK needs them first).
  - softmax denominators are normalized per head-pair *inside* the
    attention loop (reciprocal + sel2-broadcast MM + DVE mul emitted two
    heads later), removing the serial normalize block after attention.
  - out-proj / x1 / LN2 / transposes are interleaved per row block.
  - attention PSUM: psc bufs=2 (4 banks) + po/bcrt shared tag bufs=3
    (3 banks) = 7 of 8 banks.
  - attention itself is ACT-bound (exp = 16.8M elems = ~110us floor at
    1.2GHz x 128 lanes); QK/PV/exp pipeline already saturates both PE and
    ACT, so the structure is kept from v2.
"""

import sys

sys.path.insert(0, "/opt/trn_rl_repo")

import numpy as np
import ml_dtypes

BF16 = ml_dtypes.bfloat16

B, T, F, H, D, M, C = 2, 2048, 1024, 16, 64, 4, 512
NCORES = 8
R = (B * T) // NCORES  # 512 rows per core
RB = R // 128  # 4 row blocks
FT = F // 128  # 8 feature tiles
MT = (H * D) // 128  # 8 head-pair tiles
MFT = (M * F) // 128  # 32 mlp hidden tiles
KT = T // 128  # 16 key tiles
EPS = 1e-5

_CACHE = {}


def _build_nc():
    import concourse.bass as bass
    import concourse.tile as tile
    from concourse import bacc, mybir
    from concourse.masks import make_identity
    from contextlib import ExitStack

    f32 = mybir.dt.float32
    bf16 = mybir.dt.bfloat16
    AF = mybir.ActivationFunctionType
    OP = mybir.AluOpType

    nc = bacc.Bacc(
        "TRN2",
        target_bir_lowering=False,
        debug=False,
        enable_asserts=False,
        num_devices=NCORES,
    )

    def dram(name, shape, dt, **kw):
        return nc.dram_tensor(name, shape, dt, **kw).ap()

    x_d = dram("x", [R, F], f32, kind="ExternalInput")
    # host-precomputed adaLN vectors: wab = [Wa | Ba] in [2,512]-chunk rows
    wab_d = dram("wab", [2, 2 * 512], f32, kind="ExternalInput")
    # wfbf = [WfC | BfC] in column layout [128, FT] each
    wfbf_d = dram("wfbf", [128, 2 * FT], f32, kind="ExternalInput")
    # pack1: anwT(0:8) | anbT(8:16) | b1 columns(16:48), all [128, n] f32
    pack1_d = dram("pack1", [128, 48], f32, kind="ExternalInput")
    wq_d = dram("wq", [MT, 128, FT * 128], bf16, kind="ExternalInput")
    # folded-KV path inputs (replaces wkv + AllGather): see _prep_in_maps
    xT_d = dram("xT", [128, FT, T], bf16, kind="ExternalInput")
    x2T_d = dram("x2T", [128, FT, T], bf16, kind="ExternalInput")
    wg_d = dram("wg", [128, FT, 128], bf16, kind="ExternalInput")
    wstat_d = dram("wstat", [128, FT, 6], bf16, kind="ExternalInput")
    gmatk_d = dram("gmatk", [4, 64], bf16, kind="ExternalInput")
    vconst_d = dram("vconst", [4, 64], f32, kind="ExternalInput")
    sconst_d = dram("sconst", [128, 6], f32, kind="ExternalInput")
    wo_d = dram("wo", [H * D, F], bf16, kind="ExternalInput")
    wob_d = dram("wo_bias", [1, F], f32, kind="ExternalInput")
    w1_d = dram("w1", [MFT, 128, FT * 128], bf16, kind="ExternalInput")
    w2_d = dram("w2", [M * F, F], bf16, kind="ExternalInput")
    b2_d = dram("b2", [1, F], f32, kind="ExternalInput")
    selr_d = dram("selr", [2, 256], bf16, kind="ExternalInput")
    selp_d = dram("selp", [2, 128], bf16, kind="ExternalInput")
    y_d = dram("y", [R, F], f32, kind="ExternalOutput")

    groups = [[0, 1, 2, 3], [4, 5, 6, 7]]

    def bcast_row(ap_row):
        # [1, n] DRAM AP -> partition-broadcast [128, n]
        return bass.AP(
            tensor=ap_row.tensor,
            offset=ap_row.offset,
            ap=[[0, 128]] + list(ap_row.ap[-1:]),
        )

    with tile.TileContext(nc) as tc, ExitStack() as ctx:
        consts = ctx.enter_context(tc.tile_pool(name="consts", bufs=1))
        work = ctx.enter_context(tc.tile_pool(name="work", bufs=2))
        persist = ctx.enter_context(tc.tile_pool(name="persist", bufs=1))
        wstr = ctx.enter_context(tc.tile_pool(name="wstr", bufs=3))
        dramp = ctx.enter_context(tc.tile_pool(name="dramp", bufs=1, space="DRAM"))

        # ---------------- constants ----------------
        ident = consts.tile([128, 128], bf16, name="ident")
        make_identity(nc, ident)
        epst = consts.tile([128, 1], f32, name="epst")
        nc.vector.memset(epst, EPS)

        wab = consts.tile([2, 2 * 512], f32, name="wab")
        nc.sync.dma_start(out=wab, in_=wab_d)
        selr = consts.tile([2, 256], bf16, name="selr")
        nc.sync.dma_start(out=selr, in_=selr_d)
        pack1 = consts.tile([128, 48], f32, name="pack1")
        nc.gpsimd.dma_start(out=pack1, in_=pack1_d)
        anwT = pack1[:, 0:FT]
        anbT = pack1[:, FT : 2 * FT]
        b1_sb = pack1[:, 2 * FT : 2 * FT + MFT]
        wfbf = consts.tile([128, 2 * FT], f32, name="wfbf")
        nc.gpsimd.dma_start(out=wfbf, in_=wfbf_d)
        WfC = wfbf[:, 0:FT]
        BfC = wfbf[:, FT : 2 * FT]
        sconst = consts.tile([128, 6], f32, name="sconst")
        nc.sync.dma_start(out=sconst, in_=sconst_d)
        gmatk = consts.tile([4, 64], bf16, name="gmatk")
        nc.sync.dma_start(out=gmatk, in_=gmatk_d)
        # v-correction vectors broadcast down partitions: g_v, BaWn_v, sig_v, nbw_v
        vcb = consts.tile([128, 4, 64], f32, name="vcb")
        for j in range(4):
            nc.gpsimd.dma_start(out=vcb[:, j, :], in_=bcast_row(vconst_d[j : j + 1, :]))
        wob_bc = consts.tile([128, F], f32, name="wob_bc")
        nc.gpsimd.dma_start(out=wob_bc, in_=bcast_row(wob_d[0:1, :]))
        b2_bc = consts.tile([128, F], f32, name="b2_bc")
        nc.gpsimd.dma_start(out=b2_bc, in_=bcast_row(b2_d[0:1, :]))

        # phase-scoped SBUF pools (LIFO): p_big > hera > kvp; kvp closes
        # after kT/v_ext are built, hera after q-proj; mlps then reuses.
        cm_big = tc.tile_pool(name="p_big", bufs=1)
        p_big = cm_big.__enter__()
        cm_hera = tc.tile_pool(name="hera", bufs=1)
        hera = cm_hera.__enter__()
        cm_kvp = tc.tile_pool(name="kvp", bufs=1)
        kvp = cm_kvp.__enter__()

        # pre-issue the x row loads (first in the ACT DMA queue)
        x_tiles = []
        for rb in range(RB):
            x_rb = work.tile([128, F], f32, tag="x", bufs=4, name="x_rb")
            nc.scalar.dma_start(out=x_rb, in_=x_d[rb * 128 : (rb + 1) * 128, :])
            x_tiles.append(x_rb)

        # ---------------- Wa/Ba broadcast to [128, F] ----------------
        cm_ps_bc = tc.tile_pool(name="ps_bc", bufs=1, space="PSUM")
        ps_bc = cm_ps_bc.__enter__()
        wab16 = hera.tile([2, 2 * 512], bf16, name="wab16")
        with nc.allow_low_precision(reason="bf16 staging for PE broadcast"):
            nc.vector.tensor_copy(out=wab16, in_=wab)
        bc = {}
        for v, nm in ((0, "Wa_bc"), (1, "Ba_bc")):
            bt = consts.tile([128, F], f32, name=nm)
            for j in range(2):
                pb = ps_bc.tile([128, 512], f32, tag="pb", bufs=2, name="pb")
                nc.tensor.matmul(
                    pb,
                    selr[:, j * 128 : (j + 1) * 128],
                    wab16[:, v * 512 : (v + 1) * 512],
                    start=True,
                    stop=True,
                )
                nc.scalar.activation(
                    out=bt[:, j * 512 : (j + 1) * 512], in_=pb, func=AF.Copy
                )
            bc[nm] = bt
        cm_ps_bc.__exit__(None, None, None)

        # attention-era big tiles
        qT = [p_big.tile([128, R], bf16, name=f"qT{mt}") for mt in range(MT)]
        kT_lo = p_big.tile([128, T], bf16, name="kT_lo")
        kT_hi = p_big.tile([128, T], bf16, name="kT_hi")
        v_ext = [p_big.tile([128, 192], bf16, name=f"vext{kt}") for kt in range(KT)]
        out2 = [p_big.tile([128, R], bf16, name=f"out2_{hp}") for hp in range(H // 2)]
        # per-pair softmax denominators: pair p in columns [p*512:(p+1)*512],
        # even head on partition 0, odd on partition 1
        den2 = p_big.tile([2, (H // 2) * R], f32, name="den2")
        # selp broadcasts a [2,512] pair tile to [128,512]
        selp = p_big.tile([2, 128], bf16, name="selp")
        nc.gpsimd.dma_start(out=selp, in_=selp_d)
        h_res = [p_big.tile([128, F], bf16, name=f"h{rb}") for rb in range(RB)]
        x1 = [persist.tile([128, F], f32, name=f"x1_{rt}") for rt in range(RB)]
        # per-key folded-LN coefficients: [alpha, b', g', d', e'] per key tile
        abgd = p_big.tile([128, KT, 5], f32, name="abgd")

        # zero-padding memsets, off the critical path
        nc.gpsimd.memset(kT_lo[64:128, :], 0.0)
        nc.gpsimd.memset(kT_hi[0:64, :], 0.0)
        for kt in range(KT):
            eng = nc.vector if kt % 2 == 0 else nc.gpsimd
            eng.memset(v_ext[kt][:, 64:128], 0.0)
            eng.memset(v_ext[kt][:, 64:65], 1.0)

        # phase-1 tiles
        hT = [hera.tile([128, R], bf16, name=f"hT{ft}") for ft in range(FT)]

        # ================ folded-KV phase: K/V for ALL T keys ================
        # kv[key] = alpha*P[key] + beta*g + gamma*BaWn + delta*sig + nbw with
        # P = x @ Wg and per-key scalars from x/x^2 moment matmuls; no
        # cross-core AllGather (and so no exposure to core launch skew).
        wg = kvp.tile([128, FT, 128], bf16, name="wg")
        nc.sync.dma_start(out=wg, in_=wg_d)
        wstat = kvp.tile([128, FT, 6], bf16, name="wstat")
        nc.sync.dma_start(out=wstat, in_=wstat_d)

        # --- moment matmuls: stA = [1,Wa,Wa2,WaBa]^T x ; stB = [1,Wa2]^T x^2
        cm_ps_kvA = tc.tile_pool(name="ps_kvA", bufs=1, space="PSUM")
        ps_kvA = cm_ps_kvA.__enter__()
        stA = ps_kvA.tile([4, T], f32, name="stA")
        stB = ps_kvA.tile([2, T], f32, name="stB")
        for kt in range(FT):
            xs = kvp.tile([128, T], bf16, tag="xs", bufs=2, name="xs")
            nc.sync.dma_start(out=xs, in_=xT_d[:, kt, :])
            x2 = kvp.tile([128, T], bf16, tag="x2", bufs=2, name="x2t")
            nc.gpsimd.dma_start(out=x2, in_=x2T_d[:, kt, :])
            for kg in range(4):
                nc.tensor.matmul(
                    stA[:, kg * 512 : (kg + 1) * 512],
                    wstat[:, kt, 0:4],
                    xs[:, kg * 512 : (kg + 1) * 512],
                    start=(kt == 0),
                    stop=(kt == FT - 1),
                )
            for kg in range(4):
                nc.tensor.matmul(
                    stB[:, kg * 512 : (kg + 1) * 512],
                    wstat[:, kt, 4:6],
                    x2[:, kg * 512 : (kg + 1) * 512],
                    start=(kt == 0),
                    stop=(kt == FT - 1),
                )
        statsAsb = kvp.tile([4, T], bf16, name="statsAsb")
        statsBsb = kvp.tile([2, T], bf16, name="statsBsb")
        with nc.allow_low_precision(reason="bf16 staging of row moments"):
            nc.scalar.activation(out=statsAsb, in_=stA, func=AF.Copy)
            nc.scalar.activation(out=statsBsb, in_=stB, func=AF.Copy)
        cm_ps_kvA.__exit__(None, None, None)

        cm_ps_kvB = tc.tile_pool(name="ps_kvB", bufs=1, space="PSUM")
        ps_kvB = cm_ps_kvB.__enter__()

        # --- transpose moments to key-partition layout: statT[:, kb, 0:6]
        statT = kvp.tile([128, KT, 8], f32, name="statT")
        for kb in range(KT):
            aux = ps_kvB.tile([128, 512], bf16, tag="auxb", bufs=2, name="auxt")
            nc.tensor.transpose(
                aux[:, 0:4], statsAsb[:, kb * 128 : (kb + 1) * 128], ident[0:4, 0:4]
            )
            nc.tensor.transpose(
                aux[:, 4:6], statsBsb[:, kb * 128 : (kb + 1) * 128], ident[0:2, 0:2]
            )
            nc.scalar.activation(out=statT[:, kb, 0:6], in_=aux[:, 0:6], func=AF.Copy)

        # --- per-key scalar math (all keys at once via strided APs)
        V = nc.vector
        sm = kvp.tile([128, KT, 12], f32, name="sm")
        inv_F = 1.0 / float(F)
        s1 = statT[:, :, 0]
        sAc, sA2c, sABc = statT[:, :, 1], statT[:, :, 2], statT[:, :, 3]
        t1, tA2c = statT[:, :, 4], statT[:, :, 5]
        mu1, m2, var1 = sm[:, :, 0], sm[:, :, 1], sm[:, :, 2]
        tmp, tmp2_, mu2 = sm[:, :, 3], sm[:, :, 4], sm[:, :, 5]
        acc, rstd1, sq2 = sm[:, :, 6], sm[:, :, 7], sm[:, :, 8]
        vb, vg, vd = sm[:, :, 9], sm[:, :, 10], sm[:, :, 11]
        scWa, scBa = sconst[:, 0:1], sconst[:, 1:2]
        scWa2, scWaBa, scBa2 = sconst[:, 2:3], sconst[:, 3:4], sconst[:, 4:5]
        al, bp, gp, dp, ep = (abgd[:, :, j] for j in range(5))

        V.tensor_scalar(out=mu1, in0=s1, scalar1=inv_F, scalar2=None, op0=OP.mult)
        V.tensor_mul(out=m2, in0=mu1, in1=mu1)
        V.scalar_tensor_tensor(
            out=var1, in0=t1, scalar=inv_F, in1=m2, op0=OP.mult, op1=OP.subtract
        )
        nc.scalar.activation(out=gp, in_=var1, func=AF.Sqrt, bias=epst, scale=1.0)
        V.reciprocal(out=rstd1, in_=gp)
        V.tensor_scalar(out=tmp, in0=mu1, scalar1=scWa, scalar2=None, op0=OP.mult)
        V.scalar_tensor_tensor(
            out=tmp2_, in0=sAc, scalar=inv_F, in1=tmp, op0=OP.mult, op1=OP.subtract
        )
        V.tensor_mul(out=tmp2_, in0=tmp2_, in1=rstd1)
        V.tensor_scalar(out=mu2, in0=tmp2_, scalar1=scBa, scalar2=None, op0=OP.add)
        # E[h^2]
        V.tensor_mul(out=tmp, in0=mu1, in1=sA2c)
        V.tensor_scalar(
            out=acc, in0=tmp, scalar1=-2.0 * inv_F, scalar2=None, op0=OP.mult
        )
        V.scalar_tensor_tensor(
            out=acc, in0=tA2c, scalar=inv_F, in1=acc, op0=OP.mult, op1=OP.add
        )
        V.tensor_scalar(out=tmp, in0=m2, scalar1=scWa2, scalar2=None, op0=OP.mult)
        V.tensor_add(out=acc, in0=acc, in1=tmp)
        V.tensor_mul(out=acc, in0=acc, in1=rstd1)
        V.tensor_mul(out=acc, in0=acc, in1=rstd1)
        V.tensor_scalar(out=tmp, in0=mu1, scalar1=scWaBa, scalar2=None, op0=OP.mult)
        V.scalar_tensor_tensor(
            out=tmp2_, in0=sABc, scalar=inv_F, in1=tmp, op0=OP.mult, op1=OP.subtract
        )
        V.tensor_mul(out=tmp2_, in0=tmp2_, in1=rstd1)
        V.scalar_tensor_tensor(
            out=acc, in0=tmp2_, scalar=2.0, in1=acc, op0=OP.mult, op1=OP.add
        )
        # var2 = acc + mBa2 - mu2^2
        V.tensor_mul(out=tmp, in0=mu2, in1=mu2)
        V.tensor_sub(out=acc, in0=acc, in1=tmp)
        V.tensor_scalar(out=acc, in0=acc, scalar1=scBa2, scalar2=None, op0=OP.add)
        nc.scalar.activation(out=sq2, in_=acc, func=AF.Sqrt, bias=epst, scale=1.0)
        # coefficients
        V.tensor_mul(out=ep, in0=gp, in1=sq2)
        V.reciprocal(out=al, in_=ep)
        V.tensor_scalar(out=bp, in0=mu1, scalar1=-1.0, scalar2=None, op0=OP.mult)
        V.scalar_tensor_tensor(
            out=dp, in0=mu2, scalar=-1.0, in1=gp, op0=OP.mult, op1=OP.mult
        )
        # true beta/gamma/delta for the v path
        V.tensor_mul(out=vb, in0=al, in1=bp)
        V.reciprocal(out=vg, in_=sq2)
        V.tensor_mul(out=vd, in0=al, in1=dp)

        # --- transpose [b', g', d', e'] to row layout for the k-correction
        abgd16 = kvp.tile([128, KT, 4], bf16, name="abgd16")
        with nc.allow_low_precision(reason="bf16 staging for PE transpose"):
            V.tensor_copy(out=abgd16, in_=abgd[:, :, 1:5])
        browS = kvp.tile([4, T], bf16, name="browS")
        for kb in range(KT):
            aux = ps_kvB.tile([128, 512], bf16, tag="auxb", bufs=2, name="auxw")
            nc.tensor.transpose(aux[0:4, 0:128], abgd16[:, kb, :], ident)
            nc.scalar.activation(
                out=browS[:, kb * 128 : (kb + 1) * 128], in_=aux[0:4, 0:128],
                func=AF.Copy,
            )

        # --- P matmuls (second streaming pass over xT) + k/v assembly
        Pv_sb = kvp.tile([64, T], bf16, name="Pv_sb")
        Pp = {}
        for kg in range(4):
            Pp[kg] = ps_kvB.tile([128, 512], f32, tag=f"P{kg}", bufs=1, name=f"Pp{kg}")
        for kt in range(FT):
            xs = kvp.tile([128, T], bf16, tag="xs", bufs=2, name="xs2")
            eng = nc.sync if kt % 2 == 0 else nc.gpsimd
            eng.dma_start(out=xs, in_=xT_d[:, kt, :])
            for kg in range(4):
                nc.tensor.matmul(
                    Pp[kg],
                    wg[:, kt, :],
                    xs[:, kg * 512 : (kg + 1) * 512],
                    start=(kt == 0),
                    stop=False,
                )
        for kg in range(4):
            # k-correction accumulates straight into the P psum (rows 0:64)
            nc.tensor.matmul(
                Pp[kg][0:64, :],
                gmatk,
                browS[:, kg * 512 : (kg + 1) * 512],
                start=False,
                stop=True,
            )
            sl = slice(kg * 512, (kg + 1) * 512)
            with nc.allow_low_precision(reason="bf16 k values"):
                V.tensor_copy(out=kT_lo[0:64, sl], in_=Pp[kg][0:64, :])
            nc.scalar.activation(out=Pv_sb[:, sl], in_=Pp[kg][64:128, :], func=AF.Copy)
            nc.sync.dma_start(out=kT_hi[64:128, sl], in_=kT_lo[0:64, sl])

        # --- v_ext: transpose Pv to row layout, apply per-key affine
        for kt in range(KT):
            aux = ps_kvB.tile([128, 512], bf16, tag="auxb", bufs=2, name="auxv")
            nc.tensor.transpose(
                aux[:, 0:64], Pv_sb[:, kt * 128 : (kt + 1) * 128], ident[0:64, 0:64]
            )
            vc = work.tile([128, D], f32, tag="vch", bufs=2, name="vch")
            V.scalar_tensor_tensor(
                out=vc, in0=vcb[:, 0, :], scalar=sm[:, kt, 9:10], in1=vcb[:, 3, :],
                op0=OP.mult, op1=OP.add,
            )
            V.scalar_tensor_tensor(
                out=vc, in0=vcb[:, 1, :], scalar=sm[:, kt, 10:11], in1=vc,
                op0=OP.mult, op1=OP.add,
            )
            V.scalar_tensor_tensor(
                out=vc, in0=vcb[:, 2, :], scalar=sm[:, kt, 11:12], in1=vc,
                op0=OP.mult, op1=OP.add,
            )
            with nc.allow_low_precision(reason="bf16 v values"):
                V.scalar_tensor_tensor(
                    out=v_ext[kt][:, 0:D], in0=aux[:, 0:64],
                    scalar=abgd[:, kt, 0:1], in1=vc, op0=OP.mult, op1=OP.add,
                )
            V.tensor_copy(out=v_ext[kt][:, 128:192], in_=v_ext[kt][:, 0:D])
        cm_ps_kvB.__exit__(None, None, None)
        cm_kvp.__exit__(None, None, None)

        # ---------------- LN stats helper ----------------
        def ln_stats(src):
            stats = work.tile([128, 2, 6], f32, tag="stats", name="stats")
            for sg in range(2):
                nc.vector.bn_stats(
                    out=stats[:, sg, :], in_=src[:, sg * 512 : (sg + 1) * 512]
                )
            mv = work.tile([128, 2], f32, tag="mv", name="mv")
            nc.vector.bn_aggr(out=mv, in_=stats)
            rstd = work.tile([128, 1], f32, tag="rstd", name="rstd")
            nc.scalar.activation(
                out=rstd, in_=mv[:, 1:2], func=AF.Sqrt, bias=epst, scale=1.0
            )
            nc.vector.reciprocal(out=rstd, in_=rstd)
            return mv, rstd

        cm_ps1 = tc.tile_pool(name="ps1", bufs=1, space="PSUM")
        ps1 = cm_ps1.__enter__()

        # ---------------- phase 1: adaLN-1 + attn-LN + transpose (own rows) ----------------
        for rb in range(RB):
            x_rb = x_tiles[rb]
            mv1, rstd1v = ln_stats(x_rb)
            nc.vector.scalar_tensor_tensor(
                out=x_rb,
                in0=x_rb,
                scalar=mv1[:, 0:1],
                in1=bc["Wa_bc"],
                op0=OP.subtract,
                op1=OP.mult,
            )
            nc.vector.scalar_tensor_tensor(
                out=h_res[rb],
                in0=x_rb,
                scalar=rstd1v,
                in1=bc["Ba_bc"],
                op0=OP.mult,
                op1=OP.add,
            )
            mv2, rstd2v = ln_stats(h_res[rb])
            xn_bf = work.tile([128, F], bf16, tag="xn", name="xn_bf")
            nc.vector.tensor_scalar(
                out=xn_bf,
                in0=h_res[rb],
                scalar1=mv2[:, 0:1],
                scalar2=rstd2v,
                op0=OP.subtract,
                op1=OP.mult,
            )
            for ft in range(FT):
                pt = ps1.tile([128, 128], bf16, tag="tp", bufs=2, name="pt")
                nc.tensor.transpose(pt, xn_bf[:, ft * 128 : (ft + 1) * 128], ident)
                nc.scalar.activation(
                    out=hT[ft][:, rb * 128 : (rb + 1) * 128],
                    in_=pt,
                    func=AF.Identity,
                    bias=anbT[:, ft : ft + 1],
                    scale=anwT[:, ft : ft + 1],
                )

        # ---------------- phase 2: q projection ----------------
        for mt in range(MT):
            wqblk = wstr.tile([128, FT * 128], bf16, tag="wqb", bufs=3, name="wqblk")
            nc.scalar.dma_start(out=wqblk, in_=wq_d[mt])
            pq = ps1.tile([128, 512], f32, tag="sp", bufs=3, name="pq")
            for kt in range(FT):
                nc.tensor.matmul(
                    pq,
                    wqblk[:, kt * 128 : (kt + 1) * 128],
                    hT[kt],
                    start=(kt == 0),
                    stop=(kt == FT - 1),
                )
            # fold the attention 1/sqrt(D)=0.125 scale into q
            nc.scalar.activation(out=qT[mt], in_=pq, func=AF.Copy, scale=0.125)

        # fold the out-proj bias into the residual (x1 = px + (h_res + wo_b))
        for rb in range(RB):
            nc.gpsimd.tensor_add(out=h_res[rb], in0=h_res[rb], in1=wob_bc)

        # preload exp's ACT table set before attention
        warm = work.tile([1, 1], f32, tag="warm", bufs=1, name="warm")
        nc.scalar.activation(out=warm, in_=epst[0:1, 0:1], func=AF.Exp)

        cm_ps1.__exit__(None, None, None)
        cm_hera.__exit__(None, None, None)

        # mlp-era tiles reuse the kv/hera region
        cm_mlps = tc.tile_pool(name="mlps", bufs=1)
        mlps = cm_mlps.__enter__()
        h2T = [mlps.tile([128, R], bf16, name=f"h2T{ft}") for ft in range(FT)]
        g1T = [mlps.tile([128, R], bf16, name=f"g1T{mt}") for mt in range(MFT)]
        wo2sb = [mlps.tile([128, F], bf16, name=f"wo2_{hp}") for hp in range(MT)]
        for hp in range(MT):
            nc.gpsimd.dma_start(out=wo2sb[hp], in_=wo_d[hp * 128 : (hp + 1) * 128, :])

        # attention-era PSUM: psc 2x[128,1024] (4 banks) + po/bcrt tag (3)
        cm_ps_attn = tc.tile_pool(name="ps_attn", bufs=1, space="PSUM")
        ps_attn = cm_ps_attn.__enter__()

        # ---------------- phase 4: attention (per-pair normalize inline) ----------------
        def pair_norm(p):
            rcp2 = work.tile([2, 512], bf16, tag="rcp2", name="rcp2")
            with nc.allow_low_precision(reason="bf16 softmax denom broadcast"):
                nc.vector.reciprocal(
                    out=rcp2, in_=den2[:, p * 512 : (p + 1) * 512]
                )
            bcrt = ps_attn.tile([128, 512], f32, tag="po", bufs=2, name="bcrt")
            nc.tensor.matmul(bcrt, selp, rcp2, start=True, stop=True)
            nc.vector.tensor_mul(out=out2[p], in0=out2[p], in1=bcrt)

        for h in range(H):
            if h >= 4 and h % 2 == 0:
                pair_norm(h // 2 - 2)
            mt, even = h // 2, (h % 2) == 0
            kTs = kT_lo if even else kT_hi
            po = ps_attn.tile([128, 512], f32, tag="po", bufs=2, name="po")
            for c in range(KT // 2):
                psc = ps_attn.tile([128, 1024], f32, tag="ps2", bufs=3, name="psc")
                for half in range(2):
                    kt = 2 * c + half
                    nc.tensor.matmul(
                        psc[:, half * 512 : (half + 1) * 512],
                        kTs[:, kt * 128 : (kt + 1) * 128],
                        qT[mt],
                        start=True,
                        stop=True,
                    )
                pr = work.tile([128, 1024], bf16, tag="pr", bufs=4, name="pr")
                for half in range(2):
                    kt = 2 * c + half
                    nc.scalar.activation(
                        out=pr[:, half * 512 : (half + 1) * 512],
                        in_=psc[:, half * 512 : (half + 1) * 512],
                        func=AF.Exp,
                        scale=abgd[:, kt, 0:1],
                    )
                for half in range(2):
                    kt = 2 * c + half
                    lhs = v_ext[kt][:, 0:65] if even else v_ext[kt][:, 64:192]
                    outsl = po[0:65, :] if even else po[0:128, :]
                    nc.tensor.matmul(
                        outsl,
                        lhs,
                        pr[:, half * 512 : (half + 1) * 512],
                        start=(c == 0 and half == 0),
                        stop=(c == KT // 2 - 1 and half == 1),
                    )
            # stage the denominator row to SBUF (same-partition DVE copy),
            # then cross-partition SBUF->SBUF DMA into the den16 gather tile
            stg = work.tile([128, 512], f32, tag="dstg", name="dstg")
            dsl = den2[h % 2 : h % 2 + 1, mt * 512 : (mt + 1) * 512]
            if even:
                nc.vector.tensor_copy(out=stg[64:65, :], in_=po[64:65, :])
                nc.sync.dma_start(out=dsl, in_=stg[64:65, :])
                nc.vector.tensor_copy(out=out2[mt][0:64, :], in_=po[0:64, :])
            else:
                nc.vector.tensor_copy(out=stg[0:1, :], in_=po[0:1, :])
                nc.sync.dma_start(out=dsl, in_=stg[0:1, :])
                nc.vector.tensor_copy(out=out2[mt][64:128, :], in_=po[64:128, :])
        pair_norm(H // 2 - 2)
        pair_norm(H // 2 - 1)

        cm_ps_attn.__exit__(None, None, None)

        cm_ps_p5 = tc.tile_pool(name="ps_p5", bufs=1, space="PSUM")
        ps_p5 = cm_ps_p5.__enter__()

        # ---------------- phase 5+6: out proj + residual + adaLN-2, per row block ----------------
        for rt in range(RB):
            for fh in range(2):
                px = ps_p5.tile([128, 512], f32, tag="px", bufs=3, name="px")
                for hp2 in range(H // 2):
                    nc.tensor.matmul(
                        px,
                        out2[hp2][:, rt * 128 : (rt + 1) * 128],
                        wo2sb[hp2][:, fh * 512 : (fh + 1) * 512],
                        start=(hp2 == 0),
                        stop=(hp2 == H // 2 - 1),
                    )
                sl = slice(fh * 512, (fh + 1) * 512)
                nc.vector.tensor_add(out=x1[rt][:, sl], in0=px, in1=h_res[rt][:, sl])
            mv3, rstd3 = ln_stats(x1[rt])
            xn_bf = work.tile([128, F], bf16, tag="xn", name="xn2_bf")
            nc.vector.tensor_scalar(
                out=xn_bf,
                in0=x1[rt],
                scalar1=mv3[:, 0:1],
                scalar2=rstd3,
                op0=OP.subtract,
                op1=OP.mult,
            )
            # b2 folded into x1 (y = mlp2 + (x1 + b2))
            nc.gpsimd.tensor_add(out=x1[rt], in0=x1[rt], in1=b2_bc)
            for ft in range(FT):
                pt = ps_p5.tile([128, 128], bf16, tag="tp2", bufs=2, name="pt2")
                nc.tensor.transpose(pt, xn_bf[:, ft * 128 : (ft + 1) * 128], ident)
                nc.scalar.activation(
                    out=h2T[ft][:, rt * 128 : (rt + 1) * 128],
                    in_=pt,
                    func=AF.Identity,
                    bias=BfC[:, ft : ft + 1],
                    scale=WfC[:, ft : ft + 1],
                )

        cm_ps_p5.__exit__(None, None, None)

        cm_ps_mlp = tc.tile_pool(name="ps_mlp", bufs=1, space="PSUM")
        ps_mlp = cm_ps_mlp.__enter__()

        # ---------------- phase 7: mlp1 + gelu ----------------
        for mt in range(MFT):
            w1blk = wstr.tile([128, FT * 128], bf16, tag="w1b", bufs=3, name="w1blk")
            nc.gpsimd.dma_start(out=w1blk, in_=w1_d[mt])
            pg = ps_mlp.tile([128, 512], f32, tag="pg", bufs=3, name="pg")
            for kt in range(FT):
                nc.tensor.matmul(
                    pg,
                    w1blk[:, kt * 128 : (kt + 1) * 128],
                    h2T[kt],
                    start=(kt == 0),
                    stop=(kt == FT - 1),
                )
            nc.scalar.activation(
                out=g1T[mt],
                in_=pg,
                func=AF.Gelu,
                bias=b1_sb[:, mt : mt + 1],
                scale=1.0,
            )

        # ---------------- phase 8: mlp2 + residual -> y ----------------
        # mlp2's kt-th accumulation step only needs g1T[kt], so the fh=0
        # column sweep pipelines with mlp1 on the PE.
        for fh in range(2):
            pf = {}
            for rt in range(RB):
                pf[rt] = ps_mlp.tile(
                    [128, 512], f32, tag=f"pf{rt}", bufs=1, name=f"pf{rt}"
                )
            for kt in range(MFT):
                w2c = wstr.tile([128, 512], bf16, tag="w2c", bufs=6, name="w2c")
                eng = nc.gpsimd if kt % 2 == 0 else nc.sync
                eng.dma_start(
                    out=w2c, in_=w2_d[kt * 128 : (kt + 1) * 128, fh * 512 : (fh + 1) * 512]
                )
                for rt in range(RB):
                    nc.tensor.matmul(
                        pf[rt],
                        g1T[kt][:, rt * 128 : (rt + 1) * 128],
                        w2c,
                        start=(kt == 0),
                        stop=(kt == MFT - 1),
                    )
            for rt in range(RB):
                sl = slice(fh * 512, (fh + 1) * 512)
                yh = work.tile([128, 512], f32, tag="yh", bufs=2, name="yh")
                nc.vector.tensor_add(out=yh, in0=pf[rt], in1=x1[rt][:, sl])
                nc.sync.dma_start(out=y_d[rt * 128 : (rt + 1) * 128, sl], in_=yh)

        cm_ps_mlp.__exit__(None, None, None)
        cm_mlps.__exit__(None, None, None)
        cm_big.__exit__(None, None, None)

    nc.compile()
    return nc


def _prep_in_maps(inputs):
    f32 = np.float32
    wq_t = np.ascontiguousarray(
        np.asarray(inputs["wq"]).astype(BF16).reshape(FT, 128, MT, 128)
        .transpose(2, 1, 0, 3).reshape(MT, 128, FT * 128)
    )
    w1_t = np.ascontiguousarray(
        np.asarray(inputs["w1"]).astype(BF16).reshape(FT, 128, MFT, 128)
        .transpose(2, 1, 0, 3).reshape(MFT, 128, FT * 128)
    )
    selr = np.zeros((2, 256), BF16)
    selr[0, 0:128] = 1
    selr[1, 128:256] = 1
    selp = np.zeros((2, 128), BF16)
    selp[0, 0:64] = 1
    selp[1, 64:128] = 1
    # pack1: anwT | anbT | b1 columns
    pack1 = np.empty((128, 48), f32)
    pack1[:, 0:FT] = np.asarray(inputs["attn_nw"], f32).reshape(FT, 128).T
    pack1[:, FT : 2 * FT] = np.asarray(inputs["attn_nb"], f32).reshape(FT, 128).T
    pack1[:, 2 * FT :] = np.asarray(inputs["b1"], f32).reshape(MFT, 128).T

    # host-precomputed adaLN modulation vectors, per batch:
    #   g = c@gw + gb ; b = c@bw + bb
    #   Wa = nw*(1+g) ; Ba = nb*(1+g) + b
    cond = np.asarray(inputs["cond"], f32)

    def modvecs(nw, nb, gw, gb, bw, bb):
        g = cond @ np.asarray(gw, f32) + np.asarray(gb, f32)  # [B, F]
        b = cond @ np.asarray(bw, f32) + np.asarray(bb, f32)
        W = np.asarray(nw, f32) * (1.0 + g)
        Bv = np.asarray(nb, f32) * (1.0 + g) + b
        return W, Bv

    Wa, Ba = modvecs(
        inputs["amod_nw"], inputs["amod_nb"], inputs["amod_gw"],
        inputs["amod_gb"], inputs["amod_bw"], inputs["amod_bb"],
    )
    Wf, Bf = modvecs(
        inputs["fmod_nw"], inputs["fmod_nb"], inputs["fmod_gw"],
        inputs["fmod_gb"], inputs["fmod_bw"], inputs["fmod_bb"],
    )

    # folded-KV host constants, per batch
    nw = np.asarray(inputs["attn_nw"], f32)
    nb = np.asarray(inputs["attn_nb"], f32)
    wkv = np.asarray(inputs["wkv"], f32)
    Wn = nw[:, None] * wkv                      # [F, 128]
    nbw = nb @ wkv                              # [128]

    def kv_fold(b):
        wa, ba = Wa[b], Ba[b]
        Wg = wa[:, None] * Wn                   # [F, 128]
        gv = Wg.sum(0)
        sig = Wn.sum(0)
        BaWn = ba @ Wn
        wg_t = np.ascontiguousarray(
            Wg.astype(BF16).reshape(FT, 128, 128).transpose(1, 0, 2)
        )
        wstat = np.stack(
            [np.ones(F, f32), wa, wa * wa, wa * ba, np.ones(F, f32), wa * wa],
            axis=1,
        )  # [F, 6]
        wstat_t = np.ascontiguousarray(
            wstat.astype(BF16).reshape(FT, 128, 6).transpose(1, 0, 2)
        )
        gmatk = np.ascontiguousarray(
            np.stack([gv[:64], BaWn[:64], sig[:64], nbw[:64]], axis=0).astype(BF16)
        )  # [4, 64]
        vconst = np.ascontiguousarray(
            np.stack([gv[64:], BaWn[64:], sig[64:], nbw[64:]], axis=0).astype(f32)
        )  # [4, 64]
        sc = np.array(
            [wa.mean(), ba.mean(), (wa * wa).mean(), (wa * ba).mean(),
             (ba * ba).mean(), 0.0], f32,
        )
        sconst = np.ascontiguousarray(np.broadcast_to(sc, (128, 6)).astype(f32))
        return wg_t, wstat_t, gmatk, vconst, sconst

    shared = dict(
        selp=selp,
        selr=selr,
        pack1=np.ascontiguousarray(pack1),
        wq=wq_t,
        wo=np.asarray(inputs["wo"]).astype(BF16),
        wo_bias=np.asarray(inputs["wo_b"]).astype(f32).reshape(1, F),
        w1=w1_t,
        w2=np.asarray(inputs["w2"]).astype(BF16),
        b2=np.asarray(inputs["b2"]).astype(f32).reshape(1, F),
    )
    x = np.asarray(inputs["x"]).astype(f32)
    # per-batch transposed x (and x^2) for the folded-KV path
    xT_b, x2T_b, fold_b = [], [], []
    for b in range(B):
        xt = x[b].T  # [F, T]
        xT_b.append(
            np.ascontiguousarray(xt.astype(BF16).reshape(FT, 128, T).transpose(1, 0, 2))
        )
        xq = xt.astype(BF16).astype(f32)
        x2T_b.append(
            np.ascontiguousarray(
                (xq * xq).astype(BF16).reshape(FT, 128, T).transpose(1, 0, 2)
            )
        )
        fold_b.append(kv_fold(b))
    in_maps = []
    for c in range(NCORES):
        b, r0 = c // 4, (c % 4) * R
        m = dict(shared)
        m["x"] = np.ascontiguousarray(x[b, r0 : r0 + R, :])
        m["xT"] = xT_b[b]
        m["x2T"] = x2T_b[b]
        m["wg"], m["wstat"], m["gmatk"], m["vconst"], m["sconst"] = fold_b[b]
        m["wab"] = np.ascontiguousarray(
            np.concatenate(
                [Wa[b].reshape(2, 512), Ba[b].reshape(2, 512)], axis=1
            ).astype(f32)
        )
        m["wfbf"] = np.ascontiguousarray(
            np.concatenate(
                [Wf[b].reshape(FT, 128).T, Bf[b].reshape(FT, 128).T], axis=1
            ).astype(f32)
        )
        in_maps.append(m)
    return in_maps


def _run(inputs, trace=False):
    from concourse.bass_utils import run_bass_kernel_spmd

    if "nc" not in _CACHE:
        _CACHE["nc"] = _build_nc()
    nc = _CACHE["nc"]
    in_maps = _prep_in_maps(inputs)
    res = run_bass_kernel_spmd(
        nc, in_maps, core_ids=list(range(NCORES)), trace=trace
    )
    y = np.empty((B, T, F), np.float32)
    for c in range(NCORES):
        b, r0 = c // 4, (c % 4) * R
        y[b, r0 : r0 + R, :] = res.results[c]["y"]
    return y, res


def kernel(**inputs) -> np.ndarray:
    y, _ = _run(inputs, trace=False)
    return y


if __name__ == "__main__":
    _build_nc()
    print("build OK")


# revision 29
# speedup vs baseline: 1.2993x; 1.2993x over previous
"""DiT block kernel for 8x Trainium2 NeuronCores (Bass/Tile).

Sharding: row-parallel over the flattened (B,T)=4096 rows; 512 rows/core.
Cores 0-3 handle batch 0, cores 4-7 batch 1. MQA K/V is computed per-shard
and AllGather'd within each 4-core batch group. Weights are replicated and
cast to bf16; LN/residual math stays fp32.

v3 structure notes (driven by trace analysis of v2, 486us; v2 -> v3
measured 486 -> ~481us, with most of the startup win hidden behind the
AllGather's exposure to random cross-core launch skew (40-190us observed
run-to-run on this fabric) -- the post-AllGather body shrank from ~345
to ~315us):
  - the AdaLN modulation vectors (cond @ [gw|bw] + biases, folded with the
    LN weights) are precomputed on the host: the on-device mod path (wmod
    DMA + matvecs + finalize + DRAM round trip) took ~35us and gated all
    of phase 1.  Wa/Ba arrive as [2,512]-chunked rows and are broadcast to
    [128,F] with the selr PE trick; Wf/Bf arrive directly in column layout.
  - V is projected in ROW layout ([rows,64] = [keys,64] per row block)
    instead of transposed-column layout, so the post-AllGather v_ext
    assembly is plain DMAs + DVE dup copies instead of 16 serialized
    dma_start_transpose calls (~13us of fixed cost on the critical path).
    K keeps the [64,keys] layout the QK lhsT needs.  Both are packed into
    one [128,512] AllGather bounce (k rows 0:64, v row-major in 64:128).
  - kT assembly DMAs are issued before v_ext ones (QK needs them first).
  - softmax denominators are normalized per head-pair *inside* the
    attention loop (reciprocal + sel2-broadcast MM + DVE mul emitted two
    heads later), removing the serial normalize block after attention.
  - out-proj / x1 / LN2 / transposes are interleaved per row block.
  - attention PSUM: psc bufs=2 (4 banks) + po/bcrt shared tag bufs=3
    (3 banks) = 7 of 8 banks.
  - attention itself is ACT-bound (exp = 16.8M elems = ~110us floor at
    1.2GHz x 128 lanes); QK/PV/exp pipeline already saturates both PE and
    ACT, so the structure is kept from v2.
"""

import sys

sys.path.insert(0, "/opt/trn_rl_repo")

import numpy as np
import ml_dtypes

BF16 = ml_dtypes.bfloat16

B, T, F, H, D, M, C = 2, 2048, 1024, 16, 64, 4, 512
NCORES = 8
R = (B * T) // NCORES  # 512 rows per core
RB = R // 128  # 4 row blocks
FT = F // 128  # 8 feature tiles
MT = (H * D) // 128  # 8 head-pair tiles
MFT = (M * F) // 128  # 32 mlp hidden tiles
KT = T // 128  # 16 key tiles
EPS = 1e-5

_CACHE = {}


def _build_nc():
    import concourse.bass as bass
    import concourse.tile as tile
    from concourse import bacc, mybir
    from concourse.masks import make_identity
    from contextlib import ExitStack

    f32 = mybir.dt.float32
    bf16 = mybir.dt.bfloat16
    AF = mybir.ActivationFunctionType
    OP = mybir.AluOpType

    nc = bacc.Bacc(
        "TRN2",
        target_bir_lowering=False,
        debug=False,
        enable_asserts=False,
        num_devices=NCORES,
    )

    def dram(name, shape, dt, **kw):
        return nc.dram_tensor(name, shape, dt, **kw).ap()

    x_d = dram("x", [R, F], f32, kind="ExternalInput")
    # host-precomputed adaLN vectors: wab = [Wa | Ba] in [2,512]-chunk rows
    wab_d = dram("wab", [2, 2 * 512], f32, kind="ExternalInput")
    # wfbf = [WfC | BfC] in column layout [128, FT] each
    wfbf_d = dram("wfbf", [128, 2 * FT], f32, kind="ExternalInput")
    # pack1: anwT(0:8) | anbT(8:16) | b1 columns(16:48), all [128, n] f32
    pack1_d = dram("pack1", [128, 48], f32, kind="ExternalInput")
    wq_d = dram("wq", [MT, 128, FT * 128], bf16, kind="ExternalInput")
    wkv_d = dram("wkv", [128, FT, 2 * D], bf16, kind="ExternalInput")
    wo_d = dram("wo", [H * D, F], bf16, kind="ExternalInput")
    wob_d = dram("wo_bias", [1, F], f32, kind="ExternalInput")
    w1_d = dram("w1", [MFT, 128, FT * 128], bf16, kind="ExternalInput")
    w2_d = dram("w2", [M * F, F], bf16, kind="ExternalInput")
    b2_d = dram("b2", [1, F], f32, kind="ExternalInput")
    selr_d = dram("selr", [2, 256], bf16, kind="ExternalInput")
    selp_d = dram("selp", [2, 128], bf16, kind="ExternalInput")
    y_d = dram("y", [R, F], f32, kind="ExternalOutput")

    groups = [[0, 1, 2, 3], [4, 5, 6, 7]]

    def bcast_row(ap_row):
        # [1, n] DRAM AP -> partition-broadcast [128, n]
        return bass.AP(
            tensor=ap_row.tensor,
            offset=ap_row.offset,
            ap=[[0, 128]] + list(ap_row.ap[-1:]),
        )

    with tile.TileContext(nc) as tc, ExitStack() as ctx:
        consts = ctx.enter_context(tc.tile_pool(name="consts", bufs=1))
        work = ctx.enter_context(tc.tile_pool(name="work", bufs=2))
        persist = ctx.enter_context(tc.tile_pool(name="persist", bufs=1))
        wstr = ctx.enter_context(tc.tile_pool(name="wstr", bufs=3))
        dramp = ctx.enter_context(tc.tile_pool(name="dramp", bufs=1, space="DRAM"))

        # ---------------- constants ----------------
        ident = consts.tile([128, 128], bf16, name="ident")
        make_identity(nc, ident)
        epst = consts.tile([128, 1], f32, name="epst")
        nc.vector.memset(epst, EPS)

        wab = consts.tile([2, 2 * 512], f32, name="wab")
        nc.sync.dma_start(out=wab, in_=wab_d)
        selr = consts.tile([2, 256], bf16, name="selr")
        nc.sync.dma_start(out=selr, in_=selr_d)
        wkv_sb = consts.tile([128, FT, 2 * D], bf16, name="wkv_sb")
        nc.sync.dma_start(out=wkv_sb, in_=wkv_d)
        pack1 = consts.tile([128, 48], f32, name="pack1")
        nc.gpsimd.dma_start(out=pack1, in_=pack1_d)
        anwT = pack1[:, 0:FT]
        anbT = pack1[:, FT : 2 * FT]
        b1_sb = pack1[:, 2 * FT : 2 * FT + MFT]
        wfbf = consts.tile([128, 2 * FT], f32, name="wfbf")
        nc.gpsimd.dma_start(out=wfbf, in_=wfbf_d)
        WfC = wfbf[:, 0:FT]
        BfC = wfbf[:, FT : 2 * FT]
        wob_bc = consts.tile([128, F], f32, name="wob_bc")
        nc.gpsimd.dma_start(out=wob_bc, in_=bcast_row(wob_d[0:1, :]))
        b2_bc = consts.tile([128, F], f32, name="b2_bc")
        nc.gpsimd.dma_start(out=b2_bc, in_=bcast_row(b2_d[0:1, :]))

        # phase-scoped SBUF pools (LIFO): p_big > hera; mlps opens after
        # hera closes and reuses its region.
        cm_big = tc.tile_pool(name="p_big", bufs=1)
        p_big = cm_big.__enter__()
        cm_hera = tc.tile_pool(name="hera", bufs=1)
        hera = cm_hera.__enter__()

        cm_ps1 = tc.tile_pool(name="ps1", bufs=1, space="PSUM")
        ps1 = cm_ps1.__enter__()

        # ---------------- Wa/Ba broadcast to [128, F] ----------------
        wab16 = hera.tile([2, 2 * 512], bf16, name="wab16")
        with nc.allow_low_precision(reason="bf16 staging for PE broadcast"):
            nc.vector.tensor_copy(out=wab16, in_=wab)
        bc = {}
        for v, nm in ((0, "Wa_bc"), (1, "Ba_bc")):
            bt = consts.tile([128, F], f32, name=nm)
            for j in range(2):
                pb = ps1.tile([128, 512], f32, tag="sp", bufs=3, name="pb")
                nc.tensor.matmul(
                    pb,
                    selr[:, j * 128 : (j + 1) * 128],
                    wab16[:, v * 512 : (v + 1) * 512],
                    start=True,
                    stop=True,
                )
                nc.scalar.activation(
                    out=bt[:, j * 512 : (j + 1) * 512], in_=pb, func=AF.Copy
                )
            bc[nm] = bt

        # attention-era big tiles
        qT = [p_big.tile([128, R], bf16, name=f"qT{mt}") for mt in range(MT)]
        kT_lo = p_big.tile([128, T], bf16, name="kT_lo")
        kT_hi = p_big.tile([128, T], bf16, name="kT_hi")
        v_ext = [p_big.tile([128, 192], bf16, name=f"vext{kt}") for kt in range(KT)]
        out2 = [p_big.tile([128, R], bf16, name=f"out2_{hp}") for hp in range(H // 2)]
        # per-pair softmax denominators: pair p in columns [p*512:(p+1)*512],
        # even head on partition 0, odd on partition 1 (compute-engine reads
        # need base partition in {0,32,64,96}, so a [16,R] gather tile with
        # [2p:2p+2] slices is not legal)
        den2 = p_big.tile([2, (H // 2) * R], f32, name="den2")
        # selp broadcasts a [2,512] pair tile to [128,512]: rows 0:64 <- lane
        # 0 (even head), rows 64:128 <- lane 1 (odd head)
        selp = p_big.tile([2, 128], bf16, name="selp")
        nc.gpsimd.dma_start(out=selp, in_=selp_d)
        h_res = [p_big.tile([128, F], bf16, name=f"h{rb}") for rb in range(RB)]
        x1 = [persist.tile([128, F], f32, name=f"x1_{rt}") for rt in range(RB)]

        # zero-padding memsets, off the critical path
        nc.gpsimd.memset(kT_lo[64:128, :], 0.0)
        nc.gpsimd.memset(kT_hi[0:64, :], 0.0)
        for kt in range(KT):
            eng = nc.vector if kt % 2 == 0 else nc.gpsimd
            eng.memset(v_ext[kt][:, 64:128], 0.0)
            eng.memset(v_ext[kt][:, 64:65], 1.0)

        # phase-1 tiles
        hT = [hera.tile([128, R], bf16, name=f"hT{ft}") for ft in range(FT)]
        kT_loc = hera.tile([64, R], bf16, name="kT_loc")
        vR = [hera.tile([128, D], bf16, name=f"vR{rb}") for rb in range(RB)]

        # ---------------- LN stats helper ----------------
        def ln_stats(src):
            stats = work.tile([128, 2, 6], f32, tag="stats", name="stats")
            for sg in range(2):
                nc.vector.bn_stats(
                    out=stats[:, sg, :], in_=src[:, sg * 512 : (sg + 1) * 512]
                )
            mv = work.tile([128, 2], f32, tag="mv", name="mv")
            nc.vector.bn_aggr(out=mv, in_=stats)
            rstd = work.tile([128, 1], f32, tag="rstd", name="rstd")
            nc.scalar.activation(
                out=rstd, in_=mv[:, 1:2], func=AF.Sqrt, bias=epst, scale=1.0
            )
            nc.vector.reciprocal(out=rstd, in_=rstd)
            return mv, rstd

        # ---------------- phase 1: adaLN-1 + attn-LN + transpose + k/v ----------------
        for rb in range(RB):
            x_rb = work.tile([128, F], f32, tag="x", bufs=3, name="x_rb")
            nc.scalar.dma_start(out=x_rb, in_=x_d[rb * 128 : (rb + 1) * 128, :])
            mv1, rstd1 = ln_stats(x_rb)
            nc.vector.scalar_tensor_tensor(
                out=x_rb,
                in0=x_rb,
                scalar=mv1[:, 0:1],
                in1=bc["Wa_bc"],
                op0=OP.subtract,
                op1=OP.mult,
            )
            nc.vector.scalar_tensor_tensor(
                out=h_res[rb],
                in0=x_rb,
                scalar=rstd1,
                in1=bc["Ba_bc"],
                op0=OP.mult,
                op1=OP.add,
            )
            mv2, rstd2 = ln_stats(h_res[rb])
            xn_bf = work.tile([128, F], bf16, tag="xn", name="xn_bf")
            nc.vector.tensor_scalar(
                out=xn_bf,
                in0=h_res[rb],
                scalar1=mv2[:, 0:1],
                scalar2=rstd2,
                op0=OP.subtract,
                op1=OP.mult,
            )
            for ft in range(FT):
                pt = ps1.tile([128, 128], bf16, tag="tp", bufs=2, name="pt")
                nc.tensor.transpose(pt, xn_bf[:, ft * 128 : (ft + 1) * 128], ident)
                nc.scalar.activation(
                    out=hT[ft][:, rb * 128 : (rb + 1) * 128],
                    in_=pt,
                    func=AF.Identity,
                    bias=anbT[:, ft : ft + 1],
                    scale=anwT[:, ft : ft + 1],
                )
            # k projection, column layout [64, rows]
            pk = ps1.tile([64, 128], f32, tag="pk", bufs=1, name="pk")
            for kt in range(FT):
                nc.tensor.matmul(
                    pk,
                    wkv_sb[:, kt, 0:D],
                    hT[kt][:, rb * 128 : (rb + 1) * 128],
                    start=(kt == 0),
                    stop=(kt == FT - 1),
                )
            nc.scalar.activation(
                out=kT_loc[:, rb * 128 : (rb + 1) * 128], in_=pk, func=AF.Copy
            )
            # v projection, row layout [rows, 64]
            pv = ps1.tile([128, D], f32, tag="pv", bufs=1, name="pv")
            for kt in range(FT):
                nc.tensor.matmul(
                    pv,
                    hT[kt][:, rb * 128 : (rb + 1) * 128],
                    wkv_sb[:, kt, D : 2 * D],
                    start=(kt == 0),
                    stop=(kt == FT - 1),
                )
            with nc.allow_low_precision(reason="bf16 v values"):
                nc.vector.tensor_copy(out=vR[rb], in_=pv)

        # ---------------- phase 2: AllGather ASAP, then q ----------------
        kvT_bounce = dramp.tile([128, R], bf16, name="kvT_bounce")
        kvT_all = dramp.tile([4 * 128, R], bf16, name="kvT_all")
        nc.sync.dma_start(out=kvT_bounce[0:64, :], in_=kT_loc)
        for rb in range(RB):
            nc.sync.dma_start(
                out=kvT_bounce[64 + 16 * rb : 80 + 16 * rb, :].rearrange(
                    "a (b c) -> (a b) c", c=D
                ),
                in_=vR[rb],
            )
        nc.gpsimd.collective_compute(
            "AllGather",
            OP.bypass,
            replica_groups=groups,
            ins=[kvT_bounce[:, :]],
            outs=[kvT_all[:, :]],
        )

        # q projection fills the AllGather wait
        for mt in range(MT):
            wqblk = wstr.tile([128, FT * 128], bf16, tag="wqb", bufs=3, name="wqblk")
            nc.scalar.dma_start(out=wqblk, in_=wq_d[mt])
            pq = ps1.tile([128, 512], f32, tag="sp", bufs=3, name="pq")
            for kt in range(FT):
                nc.tensor.matmul(
                    pq,
                    wqblk[:, kt * 128 : (kt + 1) * 128],
                    hT[kt],
                    start=(kt == 0),
                    stop=(kt == FT - 1),
                )
            # fold the attention 1/sqrt(D)=0.125 scale into q
            nc.scalar.activation(out=qT[mt], in_=pq, func=AF.Copy, scale=0.125)

        # fold the out-proj bias into the residual while the AllGather is in
        # flight (x1 = px + (h_res + wo_b))
        for rb in range(RB):
            nc.gpsimd.tensor_add(out=h_res[rb], in0=h_res[rb], in1=wob_bc)

        # preload exp's ACT table set during the AllGather window
        warm = work.tile([1, 1], f32, tag="warm", bufs=1, name="warm")
        nc.scalar.activation(out=warm, in_=epst[0:1, 0:1], func=AF.Exp)

        cm_ps1.__exit__(None, None, None)
        cm_hera.__exit__(None, None, None)

        # mlp-era tiles reuse hera's region; wo prefetch runs in the AG window
        cm_mlps = tc.tile_pool(name="mlps", bufs=1)
        mlps = cm_mlps.__enter__()
        h2T = [mlps.tile([128, R], bf16, name=f"h2T{ft}") for ft in range(FT)]
        g1T = [mlps.tile([128, R], bf16, name=f"g1T{mt}") for mt in range(MFT)]
        wo2sb = [mlps.tile([128, F], bf16, name=f"wo2_{hp}") for hp in range(MT)]
        for hp in range(MT):
            nc.gpsimd.dma_start(out=wo2sb[hp], in_=wo_d[hp * 128 : (hp + 1) * 128, :])

        # ---------------- phase 3: kT / v_ext assembly from the AllGather ----------------
        # kT first: the QK matmuls need it before the PV ones need v_ext.
        for r in range(4):
            nc.sync.dma_start(
                out=kT_lo[0:64, r * R : (r + 1) * R],
                in_=kvT_all[r * 128 : r * 128 + 64, :],
            )
            nc.scalar.dma_start(
                out=kT_hi[64:128, r * R : (r + 1) * R],
                in_=kvT_all[r * 128 : r * 128 + 64, :],
            )
        for kt in range(KT):
            r, rb = kt // 4, kt % 4
            src = kvT_all[
                r * 128 + 64 + 16 * rb : r * 128 + 80 + 16 * rb, :
            ].rearrange("a (b c) -> (a b) c", c=D)
            eng = nc.gpsimd if kt % 2 == 0 else nc.scalar
            eng.dma_start(out=v_ext[kt][:, 0:D], in_=src)
            nc.vector.tensor_copy(out=v_ext[kt][:, 128:192], in_=v_ext[kt][:, 0:D])

        # attention-era PSUM: psc 2x[128,1024] (4 banks) + po/bcrt tag (3)
        cm_ps_attn = tc.tile_pool(name="ps_attn", bufs=1, space="PSUM")
        ps_attn = cm_ps_attn.__enter__()

        # ---------------- phase 4: attention (per-pair normalize inline) ----------------
        def pair_norm(p):
            rcp2 = work.tile([2, 512], bf16, tag="rcp2", name="rcp2")
            with nc.allow_low_precision(reason="bf16 softmax denom broadcast"):
                nc.vector.reciprocal(
                    out=rcp2, in_=den2[:, p * 512 : (p + 1) * 512]
                )
            bcrt = ps_attn.tile([128, 512], f32, tag="po", bufs=2, name="bcrt")
            nc.tensor.matmul(bcrt, selp, rcp2, start=True, stop=True)
            nc.vector.tensor_mul(out=out2[p], in0=out2[p], in1=bcrt)

        for h in range(H):
            if h >= 4 and h % 2 == 0:
                pair_norm(h // 2 - 2)
            mt, even = h // 2, (h % 2) == 0
            kTs = kT_lo if even else kT_hi
            po = ps_attn.tile([128, 512], f32, tag="po", bufs=2, name="po")
            for c in range(KT // 2):
                psc = ps_attn.tile([128, 1024], f32, tag="ps2", bufs=3, name="psc")
                for half in range(2):
                    kt = 2 * c + half
                    nc.tensor.matmul(
                        psc[:, half * 512 : (half + 1) * 512],
                        kTs[:, kt * 128 : (kt + 1) * 128],
                        qT[mt],
                        start=True,
                        stop=True,
                    )
                pr = work.tile([128, 1024], bf16, tag="pr", bufs=4, name="pr")
                nc.scalar.activation(out=pr, in_=psc, func=AF.Exp)
                for half in range(2):
                    kt = 2 * c + half
                    lhs = v_ext[kt][:, 0:65] if even else v_ext[kt][:, 64:192]
                    outsl = po[0:65, :] if even else po[0:128, :]
                    nc.tensor.matmul(
                        outsl,
                        lhs,
                        pr[:, half * 512 : (half + 1) * 512],
                        start=(c == 0 and half == 0),
                        stop=(c == KT // 2 - 1 and half == 1),
                    )
            # stage the denominator row to SBUF (same-partition DVE copy),
            # then cross-partition SBUF->SBUF DMA into the den16 gather tile
            stg = work.tile([128, 512], f32, tag="dstg", name="dstg")
            dsl = den2[h % 2 : h % 2 + 1, mt * 512 : (mt + 1) * 512]
            if even:
                nc.vector.tensor_copy(out=stg[64:65, :], in_=po[64:65, :])
                nc.sync.dma_start(out=dsl, in_=stg[64:65, :])
                nc.vector.tensor_copy(out=out2[mt][0:64, :], in_=po[0:64, :])
            else:
                nc.vector.tensor_copy(out=stg[0:1, :], in_=po[0:1, :])
                nc.sync.dma_start(out=dsl, in_=stg[0:1, :])
                nc.vector.tensor_copy(out=out2[mt][64:128, :], in_=po[64:128, :])
        pair_norm(H // 2 - 2)
        pair_norm(H // 2 - 1)

        cm_ps_attn.__exit__(None, None, None)

        cm_ps_p5 = tc.tile_pool(name="ps_p5", bufs=1, space="PSUM")
        ps_p5 = cm_ps_p5.__enter__()

        # ---------------- phase 5+6: out proj + residual + adaLN-2, per row block ----------------
        for rt in range(RB):
            for fh in range(2):
                px = ps_p5.tile([128, 512], f32, tag="px", bufs=3, name="px")
                for hp2 in range(H // 2):
                    nc.tensor.matmul(
                        px,
                        out2[hp2][:, rt * 128 : (rt + 1) * 128],
                        wo2sb[hp2][:, fh * 512 : (fh + 1) * 512],
                        start=(hp2 == 0),
                        stop=(hp2 == H // 2 - 1),
                    )
                sl = slice(fh * 512, (fh + 1) * 512)
                nc.vector.tensor_add(out=x1[rt][:, sl], in0=px, in1=h_res[rt][:, sl])
            mv3, rstd3 = ln_stats(x1[rt])
            xn_bf = work.tile([128, F], bf16, tag="xn", name="xn2_bf")
            nc.vector.tensor_scalar(
                out=xn_bf,
                in0=x1[rt],
                scalar1=mv3[:, 0:1],
                scalar2=rstd3,
                op0=OP.subtract,
                op1=OP.mult,
            )
            # b2 folded into x1 (y = mlp2 + (x1 + b2))
            nc.gpsimd.tensor_add(out=x1[rt], in0=x1[rt], in1=b2_bc)
            for ft in range(FT):
                pt = ps_p5.tile([128, 128], bf16, tag="tp2", bufs=2, name="pt2")
                nc.tensor.transpose(pt, xn_bf[:, ft * 128 : (ft + 1) * 128], ident)
                nc.scalar.activation(
                    out=h2T[ft][:, rt * 128 : (rt + 1) * 128],
                    in_=pt,
                    func=AF.Identity,
                    bias=BfC[:, ft : ft + 1],
                    scale=WfC[:, ft : ft + 1],
                )

        cm_ps_p5.__exit__(None, None, None)

        cm_ps_mlp = tc.tile_pool(name="ps_mlp", bufs=1, space="PSUM")
        ps_mlp = cm_ps_mlp.__enter__()

        # ---------------- phase 7: mlp1 + gelu ----------------
        for mt in range(MFT):
            w1blk = wstr.tile([128, FT * 128], bf16, tag="w1b", bufs=3, name="w1blk")
            nc.gpsimd.dma_start(out=w1blk, in_=w1_d[mt])
            pg = ps_mlp.tile([128, 512], f32, tag="pg", bufs=3, name="pg")
            for kt in range(FT):
                nc.tensor.matmul(
                    pg,
                    w1blk[:, kt * 128 : (kt + 1) * 128],
                    h2T[kt],
                    start=(kt == 0),
                    stop=(kt == FT - 1),
                )
            nc.scalar.activation(
                out=g1T[mt],
                in_=pg,
                func=AF.Gelu,
                bias=b1_sb[:, mt : mt + 1],
                scale=1.0,
            )

        # ---------------- phase 8: mlp2 + residual -> y ----------------
        # mlp2's kt-th accumulation step only needs g1T[kt], so the fh=0
        # column sweep pipelines with mlp1 on the PE.
        for fh in range(2):
            pf = {}
            for rt in range(RB):
                pf[rt] = ps_mlp.tile(
                    [128, 512], f32, tag=f"pf{rt}", bufs=1, name=f"pf{rt}"
                )
            for kt in range(MFT):
                w2c = wstr.tile([128, 512], bf16, tag="w2c", bufs=6, name="w2c")
                eng = nc.gpsimd if kt % 2 == 0 else nc.sync
                eng.dma_start(
                    out=w2c, in_=w2_d[kt * 128 : (kt + 1) * 128, fh * 512 : (fh + 1) * 512]
                )
                for rt in range(RB):
                    nc.tensor.matmul(
                        pf[rt],
                        g1T[kt][:, rt * 128 : (rt + 1) * 128],
                        w2c,
                        start=(kt == 0),
                        stop=(kt == MFT - 1),
                    )
            for rt in range(RB):
                sl = slice(fh * 512, (fh + 1) * 512)
                yh = work.tile([128, 512], f32, tag="yh", bufs=2, name="yh")
                nc.vector.tensor_add(out=yh, in0=pf[rt], in1=x1[rt][:, sl])
                nc.sync.dma_start(out=y_d[rt * 128 : (rt + 1) * 128, sl], in_=yh)

        cm_ps_mlp.__exit__(None, None, None)
        cm_mlps.__exit__(None, None, None)
        cm_big.__exit__(None, None, None)

    nc.compile()
    return nc


def _prep_in_maps(inputs):
    f32 = np.float32
    wq_t = np.ascontiguousarray(
        np.asarray(inputs["wq"]).astype(BF16).reshape(FT, 128, MT, 128)
        .transpose(2, 1, 0, 3).reshape(MT, 128, FT * 128)
    )
    w1_t = np.ascontiguousarray(
        np.asarray(inputs["w1"]).astype(BF16).reshape(FT, 128, MFT, 128)
        .transpose(2, 1, 0, 3).reshape(MFT, 128, FT * 128)
    )
    selr = np.zeros((2, 256), BF16)
    selr[0, 0:128] = 1
    selr[1, 128:256] = 1
    selp = np.zeros((2, 128), BF16)
    selp[0, 0:64] = 1
    selp[1, 64:128] = 1
    # pack1: anwT | anbT | b1 columns
    pack1 = np.empty((128, 48), f32)
    pack1[:, 0:FT] = np.asarray(inputs["attn_nw"], f32).reshape(FT, 128).T
    pack1[:, FT : 2 * FT] = np.asarray(inputs["attn_nb"], f32).reshape(FT, 128).T
    pack1[:, 2 * FT :] = np.asarray(inputs["b1"], f32).reshape(MFT, 128).T

    # host-precomputed adaLN modulation vectors, per batch:
    #   g = c@gw + gb ; b = c@bw + bb
    #   Wa = nw*(1+g) ; Ba = nb*(1+g) + b
    cond = np.asarray(inputs["cond"], f32)

    def modvecs(nw, nb, gw, gb, bw, bb):
        g = cond @ np.asarray(gw, f32) + np.asarray(gb, f32)  # [B, F]
        b = cond @ np.asarray(bw, f32) + np.asarray(bb, f32)
        W = np.asarray(nw, f32) * (1.0 + g)
        Bv = np.asarray(nb, f32) * (1.0 + g) + b
        return W, Bv

    Wa, Ba = modvecs(
        inputs["amod_nw"], inputs["amod_nb"], inputs["amod_gw"],
        inputs["amod_gb"], inputs["amod_bw"], inputs["amod_bb"],
    )
    Wf, Bf = modvecs(
        inputs["fmod_nw"], inputs["fmod_nb"], inputs["fmod_gw"],
        inputs["fmod_gb"], inputs["fmod_bw"], inputs["fmod_bb"],
    )

    shared = dict(
        selp=selp,
        selr=selr,
        pack1=np.ascontiguousarray(pack1),
        wq=wq_t,
        wkv=np.ascontiguousarray(
            np.asarray(inputs["wkv"]).astype(BF16).reshape(FT, 128, 2 * D)
            .transpose(1, 0, 2)
        ),
        wo=np.asarray(inputs["wo"]).astype(BF16),
        wo_bias=np.asarray(inputs["wo_b"]).astype(f32).reshape(1, F),
        w1=w1_t,
        w2=np.asarray(inputs["w2"]).astype(BF16),
        b2=np.asarray(inputs["b2"]).astype(f32).reshape(1, F),
    )
    x = np.asarray(inputs["x"]).astype(f32)
    in_maps = []
    for c in range(NCORES):
        b, r0 = c // 4, (c % 4) * R
        m = dict(shared)
        m["x"] = np.ascontiguousarray(x[b, r0 : r0 + R, :])
        m["wab"] = np.ascontiguousarray(
            np.concatenate(
                [Wa[b].reshape(2, 512), Ba[b].reshape(2, 512)], axis=1
            ).astype(f32)
        )
        m["wfbf"] = np.ascontiguousarray(
            np.concatenate(
                [Wf[b].reshape(FT, 128).T, Bf[b].reshape(FT, 128).T], axis=1
            ).astype(f32)
        )
        in_maps.append(m)
    return in_maps


def _run(inputs, trace=False):
    from concourse.bass_utils import run_bass_kernel_spmd

    if "nc" not in _CACHE:
        _CACHE["nc"] = _build_nc()
    nc = _CACHE["nc"]
    in_maps = _prep_in_maps(inputs)
    res = run_bass_kernel_spmd(
        nc, in_maps, core_ids=list(range(NCORES)), trace=trace
    )
    y = np.empty((B, T, F), np.float32)
    for c in range(NCORES):
        b, r0 = c // 4, (c % 4) * R
        y[b, r0 : r0 + R, :] = res.results[c]["y"]
    return y, res


def kernel(**inputs) -> np.ndarray:
    y, _ = _run(inputs, trace=False)
    return y


if __name__ == "__main__":
    _build_nc()
    print("build OK")
